# revision 47
# baseline (speedup 1.0000x reference)
"""Trainium2 Bass kernel for nn_MetricLoss (segment_reduce / discriminative loss).

Reference math (B=4 samples, K=32 labels, D=16, H=W=384):
  cents[s,k,:]  = mean of embeddings of sample s where label==k
  push[s]       = sum_{k<j} relu(0.25 - L1(c_sk, c_sj))^2 / 496
  pull[s]       = mean over ALL B*H*W pixels p of  L1(e_p, c_s,label_p)^2
  loss          = mean_s (push[s] + 0.1 * pull[s])

v2 design — host sorts pixels by label so that the centroid needed by any
on-chip operation is a per-partition constant; this removes every gather.

  Launch A (centroid partial sums; "lane" transposed layout):
    partition p = 16*j + d holds the d-th embedding component of pixels whose
    label k has k%8 == j, grouped into 16 equal segments (q=k//8, b) on the
    free axis (zero padded).  Per 4-segment chunk: two 2x tensor_tensor
    halvings + a short 1x tensor_reduce -> partial sums per (b,k,d).  Host
    reduces over cores, forms centroids, computes push exactly in f64.

  Launch B (pull term; pixel-partition d-major layout, abs-free):
    partition p holds pixels of label k(p) = p//4; emb2[p, d, t] is d-major.
    Uses |x| = x - 2*min(x,0):
      dist_s = (sum_d e - sum_d c_s) - 2 * sum_d min(e_d - c_sd, 0)
    min-diffs: one fused tensor_scalar(subtract, min 0) per (s,d) on DVE at
    4x (rows < N_DVE) or ACT relu(c - e) = -min (rest); a 4-level 2x adder
    tree (signs resolved by add/sub per the pair plan); t1 = Tm - ec2_s
    with ec2 = (sum_d e - sum_d c_s)/2 precomputed on host; ACT Square;
    bf16 s-compress + 1x reduce -> pacc[p] = sum_{s,t} (dist/2)^2.
    Host: pull = (4*sum pacc - exact zero-pad correction) / N.
    No ACT/Pool accumulators anywhere (their drain costs ~6us per launch).
"""

import numpy as np
import ml_dtypes

import concourse.bass as bass
import concourse.bacc as bacc
import concourse.mybir as mybir
from concourse.tile import TileContext
from concourse.bass_utils import run_bass_kernel_spmd

BF16 = ml_dtypes.bfloat16
F32 = np.float32

# problem constants (hardcoded per contract)
B, H, W, D, K = 4, 384, 384, 16, 32
NCORES = 8
NPIX_TOT = B * H * W            # 589824
P = 128                         # partitions

PUSH_MARGIN = 0.25
PUSH_W = 1.0
PULL_W = 0.1
NCMP = K * (K - 1) / 2.0

_built = {}


# --------------------------------------------------------------------------
# device programs
# --------------------------------------------------------------------------

def _build_a(SbA):
    """Centroid partial sums.  embA [128=(16j+d), 16*SbA] bf16 -> accA [128,16] f32."""
    nc = bacc.Bacc("TRN2", target_bir_lowering=False, debug=False)
    bf = mybir.dt.bfloat16
    f32 = mybir.dt.float32
    FREE = 16 * SbA

    embA_d = nc.dram_tensor("embA", [P, FREE], bf, kind="ExternalInput")
    accA_d = nc.dram_tensor("accA", [P, 16], f32, kind="ExternalOutput")

    with TileContext(nc) as tc:
        with (
            tc.tile_pool(name="sbuf", bufs=1) as pool,
            tc.tile_pool(name="scr", bufs=2) as spool,
        ):
            chunks = [pool.tile([P, 4, SbA], bf, name=f"embc{i}") for i in range(4)]
            accA = pool.tile([P, 16], f32)
            src = embA_d.ap().rearrange("p (s t) -> p s t", t=SbA)
            dma_engines = [nc.sync, nc.scalar, nc.sync, nc.scalar]
            for c in range(4):
                dma_engines[c].dma_start(
                    out=chunks[c][:], in_=src[:, 4 * c : 4 * c + 4, :]
                )
            # segment sums per chunk: two 2x tensor_tensor halving levels,
            # then a short 1x tensor_reduce (cheaper than one full 1x reduce).
            # No ACT/Pool accumulators — their use costs fixed drain time.
            h = SbA // 2
            for c in range(4):
                t1 = spool.tile([P, 4, h], bf, tag="t1")
                nc.vector.tensor_tensor(
                    out=t1[:], in0=chunks[c][:, :, 0:h], in1=chunks[c][:, :, h:SbA],
                    op=mybir.AluOpType.add,
                )
                t2 = spool.tile([P, 4, h // 2], bf, tag="t2")
                nc.vector.tensor_tensor(
                    out=t2[:], in0=t1[:, :, 0 : h // 2], in1=t1[:, :, h // 2 : h],
                    op=mybir.AluOpType.add,
                )
                nc.vector.tensor_reduce(
                    out=accA[:, 4 * c : 4 * c + 4],
                    in_=t2[:],
                    axis=mybir.AxisListType.X,
                    op=mybir.AluOpType.add,
                )
            nc.sync.dma_start(out=accA_d.ap(), in_=accA[:])
    nc.compile()
    return nc


N_DVE = 7  # min-units per s on DVE (rows 0..N_DVE-1); rest on ACT


def _tree_plan():
    """Pairing plan for the 16 -> 1 adder tree under mixed row types.
    Rows 0..N_DVE-1 are 'm' (min(e-c,0) <= 0), the rest 'q' (= -m >= 0).
    Each level pairs row i with row i+L/2; subtract when types differ.
    Returns per-level lists of (lo, hi, op, out_type) groups, consecutive
    same-op pairs merged."""
    types = ["m"] * N_DVE + ["q"] * (16 - N_DVE)
    levels = []
    while len(types) > 1:
        half = len(types) // 2
        pairs = [
            (i, i + half,
             "sub" if types[i] != types[i + half] else "add",
             types[i])
            for i in range(half)
        ]
        groups = []
        for pr in pairs:
            if groups and groups[-1][2] == pr[2] and groups[-1][1] == pr[0]:
                lo0, _, op, n = groups[-1]
                groups[-1] = (lo0, pr[0] + 1, op, n + 1)
            else:
                groups.append((pr[0], pr[0] + 1, pr[2], 1))
        levels.append((half, groups))
        types = [p[3] for p in pairs]
    assert types[0] == "m"
    return levels


def _build_b(S):
    """Pull term.  emb2 [128, 16*S] bf16 (d-major), ec2 [128, 4*S] bf16
    (host-precomputed (sum_d e - sum_d c_s)/2 per s), centq [128,64] f32
    (c per (s,d)) -> pacc [128, 1] f32.

    Uses |x| = x - 2*min(x,0):
      dist_s = (sum_d e - sum_d c) - 2 * sum_d min(e_d - c_d, 0)
    min-diffs on DVE (ts subtract+min at 4x) and ACT (relu(c-e) = -min);
    adder tree on DVE with signs resolved by add/sub; t1 = Tm - ec2_s
    = -dist/2; ACT Square; s-compress; single 1x reduce -> pacc [128,1]."""
    nc = bacc.Bacc("TRN2", target_bir_lowering=False, debug=False)
    bf = mybir.dt.bfloat16
    f32 = mybir.dt.float32

    emb2_d = nc.dram_tensor("emb2", [P, 16 * S], bf, kind="ExternalInput")
    ec2_d = nc.dram_tensor("ec2", [P, 4 * S], bf, kind="ExternalInput")
    centq_d = nc.dram_tensor("centq", [P, 64], f32, kind="ExternalInput")
    pacc_d = nc.dram_tensor("pacc", [P, 1], f32, kind="ExternalOutput")
    plan = _tree_plan()

    with TileContext(nc) as tc:
        with (
            tc.tile_pool(name="sbuf", bufs=1) as pool,
            tc.tile_pool(name="apool", bufs=3) as apool,
            tc.tile_pool(name="hpool", bufs=2) as hpool,
            tc.tile_pool(name="tpool", bufs=4) as tpool,
            tc.tile_pool(name="spool", bufs=2) as spool,
        ):
            # separate chunk tiles so unit deps are DMA-chunk precise
            chunks = [pool.tile([P, 4, S], bf, name=f"embc{i}") for i in range(4)]
            ec2 = pool.tile([P, 4, S], bf)
            centq = pool.tile([P, 64], f32)
            pacc = pool.tile([P, 1], f32)

            nc.sync.dma_start(out=centq[:], in_=centq_d.ap())
            src = emb2_d.ap().rearrange("p (d t) -> p d t", t=S)
            # chunks on two DGE queues -> parallel transfers (no gpsimd:
            # touching Pool costs ~2.7us of extra drain at NEFF end)
            dma_engines = [nc.sync, nc.scalar, nc.sync, nc.scalar]
            for c in range(4):
                dma_engines[c].dma_start(
                    out=chunks[c][:], in_=src[:, 4 * c : 4 * c + 4, :]
                )
            nc.scalar.dma_start(
                out=ec2[:], in_=ec2_d.ap().rearrange("p (s t) -> p s t", t=S)
            )

            sq_jobs = []
            for s in range(4):
                a_s = apool.tile([P, 16, S], bf, tag="a")
                for d in range(16):
                    col = 16 * s + d
                    src_row = chunks[d // 4][:, d % 4, :]
                    if d < N_DVE:  # DVE: m_d = min(e-c, 0)
                        nc.vector.tensor_scalar(
                            out=a_s[:, d, :],
                            in0=src_row,
                            scalar1=centq[:, col : col + 1],
                            scalar2=0.0,
                            op0=mybir.AluOpType.subtract,
                            op1=mybir.AluOpType.min,
                        )
                    else:  # ACT: q_d = relu(c-e) = -m_d
                        nc.scalar.activation(
                            out=a_s[:, d, :],
                            in_=src_row,
                            func=mybir.ActivationFunctionType.Relu,
                            bias=centq[:, col : col + 1],
                            scale=-1.0,
                        )
                # adder tree on DVE (2x tensor_tensor), per the pair plan
                cur = a_s
                for li, (half, groups) in enumerate(plan):
                    nxt = hpool.tile([P, half, S], bf, tag=f"h{li}")
                    for lo, hi, op, _n in groups:
                        nc.vector.tensor_tensor(
                            out=nxt[:, lo:hi, :] if half > 1 else nxt[:],
                            in0=cur[:, lo:hi, :],
                            in1=cur[:, lo + half : hi + half, :],
                            op=(mybir.AluOpType.subtract if op == "sub"
                                else mybir.AluOpType.add),
                        )
                    cur = nxt
                # t1 = Tm - ec2_s = -dist/2
                t1 = tpool.tile([P, S], bf, tag="t1")
                nc.vector.tensor_tensor(
                    out=t1[:], in0=cur[:], in1=ec2[:, s, :],
                    op=mybir.AluOpType.subtract,
                )
                sq_jobs.append((s, t1))
            # sq_s = t1^2 on ACT (no accumulator — its readout costs ~6us of
            # fixed drain); emitted after all unit work so the in-order ACT
            # queue never stalls on a pending tree.  dist^2 = 4*sq (host
            # applies the factor 4).
            sq4 = pool.tile([P, 4, S], bf)
            for s, t1 in sq_jobs:
                nc.scalar.activation(
                    out=sq4[:, s, :],
                    in_=t1[:],
                    func=mybir.ActivationFunctionType.Square,
                )
            # only sum_s pull is needed by the loss -> compress across s
            # with 2x adds before the 1x reduce
            u1 = pool.tile([P, 2, S], bf)
            nc.vector.tensor_tensor(
                out=u1[:], in0=sq4[:, 0:2, :], in1=sq4[:, 2:4, :],
                op=mybir.AluOpType.add,
            )
            u2 = pool.tile([P, 1, S], bf)
            nc.vector.tensor_tensor(
                out=u2[:], in0=u1[:, 0:1, :], in1=u1[:, 1:2, :],
                op=mybir.AluOpType.add,
            )
            nc.vector.tensor_reduce(
                out=pacc[:, 0:1],
                in_=u2[:, 0, :],
                axis=mybir.AxisListType.X,
                op=mybir.AluOpType.add,
            )
            nc.sync.dma_start(out=pacc_d.ap(), in_=pacc[:])
    nc.compile()
    return nc


def _get(name, param):
    key = (name, param)
    if key not in _built:
        _built[key] = _build_a(param) if name == "A" else _build_b(param)
    return _built[key]


# --------------------------------------------------------------------------
# host-side layout / prep
# --------------------------------------------------------------------------

def _round_up(x, m):
    return ((x + m - 1) // m) * m


def _split_shares(cnt):
    """Even split of cnt items over NCORES: list of per-core counts."""
    base, rem = divmod(int(cnt), NCORES)
    return [base + (1 if c < rem else 0) for c in range(NCORES)]


def _prep_layouts(emb_flat, lab_flat):
    """Build both device layouts + all bookkeeping from the raw inputs."""
    order = np.argsort(lab_flat, kind="stable")  # label-major; index (thus b) asc
    cnt_k = np.bincount(lab_flat, minlength=K)
    b_of = (np.arange(NPIX_TOT) // (H * W)).astype(np.int64)
    comb = lab_flat.astype(np.int64) * B + b_of  # label-major, b-minor == order
    cnt_kb = np.bincount(comb, minlength=K * B).reshape(K, B)  # [k, b]

    emb_bf = emb_flat.astype(BF16)

    # ---- launch B layout: partition 4k+r, d-major ----
    sharesB = {k: _split_shares(cnt_k[k]) for k in range(K)}
    maxshareB = max(max(v) for v in sharesB.values())
    S = max(_round_up(_round_up(maxshareB, 4) // 4, 16), 64)
    emb2 = np.zeros((NCORES, P, D, S), dtype=BF16)
    npad = np.zeros((NCORES, K), dtype=np.int64)
    k_starts = np.concatenate([[0], np.cumsum(cnt_k)])
    for k in range(K):
        blk = order[k_starts[k] : k_starts[k + 1]]
        off = 0
        for c in range(NCORES):
            n = sharesB[k][c]
            npad[c, k] = 4 * S - n
            if n == 0:
                continue
            arr = emb_bf[blk[off : off + n]]  # [n, D]
            off += n
            buf = np.zeros((4 * S, D), dtype=BF16)
            buf[:n] = arr
            emb2[c, 4 * k : 4 * k + 4] = buf.reshape(4, S, D).transpose(0, 2, 1)
    E2f = emb2.astype(np.float32).sum(axis=2) * 0.5  # [8, 128, S] f32

    # ---- launch A layout: partition 16j+d, segments (q,b) ----
    sharesA = np.zeros((K, B, NCORES), dtype=np.int64)
    for k in range(K):
        for b in range(B):
            sharesA[k, b] = _split_shares(cnt_kb[k, b])
    maxshareA = int(sharesA.max())
    SbA = max(_round_up(maxshareA, 16), 32)
    embA = np.zeros((NCORES, P, 16 * SbA), dtype=BF16)
    kb_starts = np.concatenate([[0], np.cumsum(cnt_kb.reshape(-1))])
    for k in range(K):
        j, q = k % 8, k // 8
        for b in range(B):
            blk = order[kb_starts[k * B + b] : kb_starts[k * B + b + 1]]
            seg = 4 * q + b
            off = 0
            for c in range(NCORES):
                n = sharesA[k, b, c]
                if n == 0:
                    continue
                arr = emb_bf[blk[off : off + n]]  # [n, D]
                off += n
                embA[c, 16 * j : 16 * j + 16, seg * SbA : seg * SbA + n] = arr.T
    return dict(
        S=S, SbA=SbA, npad=npad, cnt_kb=cnt_kb,
        embA=embA, emb2=emb2, E2f=E2f,
    )


def _reduce_a(results, L):
    """accA [8][128,16] -> cents [B,K,D] f64 (+ f32 copy)."""
    acc = np.zeros((P, 16), dtype=np.float64)
    for c in range(NCORES):
        acc += results[c]["accA"].astype(np.float64)
    arr = acc.reshape(8, 16, 4, 4)  # [j, d, q, b]
    sums = arr.transpose(3, 2, 0, 1).reshape(B, K, D)  # [b, (q,j)=k, d]
    cnt = L["cnt_kb"].T.astype(np.float64)  # [b, k]
    cents = np.where(
        cnt[:, :, None] > 0, sums / np.maximum(cnt, 1.0)[:, :, None], 0.0
    )
    return cents


def _prep_b_smalls(cents32, E2f):
    """centq [128,64] f32 (c per (s,d)) and per-core ec2 [128, 4, S] bf16
    (= (sum_d e - sum_d c_s)/2)."""
    kk = np.arange(P) // 4  # label of partition
    centq = np.zeros((P, 64), dtype=np.float32)
    csum2 = np.zeros((P, 4), dtype=np.float32)
    for s in range(4):
        centq[:, 16 * s : 16 * s + 16] = cents32[s][kk]  # [128, 16]
        csum2[:, s] = cents32[s][kk].sum(-1) * 0.5
    ec2 = (E2f[:, :, None, :] - csum2[None, :, :, None]).astype(BF16)  # [8,128,4,S]
    return centq, csum2, ec2


def _push_host(cents):
    """Exact push term per sample from centroids (f64)."""
    dmat = np.abs(cents[:, :, None, :] - cents[:, None, :, :]).sum(-1)  # [B,K,K]
    marg = np.maximum(PUSH_MARGIN - dmat, 0.0)
    iu = np.triu_indices(K, k=1)
    return np.array([(marg[s][iu] ** 2).sum() / NCMP for s in range(B)])


def _pad_correction(cents32, csum2_sk, npad):
    """Exact removal of zero-pad slots' contribution (e == 0), replicating the
    device's min-trick bf16 pipeline (same tree plan) step by step."""
    c = cents32                                            # [s, k, 16] f32
    a = np.empty_like(c)
    a[..., :N_DVE] = np.minimum(0.0 - c[..., :N_DVE], 0.0)  # DVE rows: m
    a[..., N_DVE:] = np.maximum(c[..., N_DVE:], 0.0)        # ACT rows: q = -m
    cur = a.astype(BF16)
    for half, groups in _tree_plan():
        nxt = np.zeros(c.shape[:2] + (half,), dtype=BF16)
        for lo, hi, op, _n in groups:
            x0 = cur[..., lo:hi].astype(np.float32)
            x1 = cur[..., lo + half : hi + half].astype(np.float32)
            nxt[..., lo:hi] = (x0 - x1 if op == "sub" else x0 + x1).astype(BF16)
        cur = nxt
    tm = cur[..., 0]                                       # [s, k] bf16
    # t1 = tm - ec2_pad  with  ec2_pad = bf16(0 - csum2);
    # device: sq = bf16(t1^2), then bf16 pairwise s-compress; dist^2 = 4*sq
    ec2_pad = (-csum2_sk).astype(BF16)
    t1 = (tm.astype(np.float32) - ec2_pad.astype(np.float32)).astype(BF16)
    sq = (t1.astype(np.float32) ** 2).astype(BF16)         # [s, k]
    u1 = (sq[0:2].astype(np.float32) + sq[2:4].astype(np.float32)).astype(BF16)
    u2 = (u1[0].astype(np.float32) + u1[1].astype(np.float32)).astype(BF16)
    padtot = npad.sum(axis=0).astype(np.float64)           # [k]
    return 4.0 * (u2.astype(np.float64) * padtot).sum()    # scalar: sum over s


# --------------------------------------------------------------------------
# orchestration
# --------------------------------------------------------------------------

def run_launches(embeddings, labels, trace=False, trace_kwargs=None):
    emb_flat = np.ascontiguousarray(np.asarray(embeddings), dtype=F32).reshape(
        NPIX_TOT, D
    )
    lab_flat = np.ascontiguousarray(np.asarray(labels), dtype=np.int32).reshape(
        NPIX_TOT
    )
    L = _prep_layouts(emb_flat, lab_flat)
    core_ids = list(range(NCORES))
    kw = dict(trace=trace, **(trace_kwargs or {}))

    in_a = [{"embA": L["embA"][c]} for c in core_ids]
    resA = run_bass_kernel_spmd(_get("A", L["SbA"]), in_a, core_ids, **kw)
    cents = _reduce_a(resA.results, L)
    cents32 = cents.astype(np.float32)

    centq, csum2, ec2 = _prep_b_smalls(cents32, L["E2f"])
    in_b = [
        {
            "emb2": L["emb2"][c].reshape(P, 16 * L["S"]),
            "ec2": ec2[c].reshape(P, 4 * L["S"]),
            "centq": centq,
        }
        for c in core_ids
    ]
    resB = run_bass_kernel_spmd(_get("B", L["S"]), in_b, core_ids, **kw)

    dev = 0.0
    for c in core_ids:
        dev += float(resB.results[c]["pacc"].astype(np.float64).sum())
    dev *= 4.0  # device returns sum over s of (dist/2)^2
    csum2_sk = cents32.sum(-1) * 0.5  # [s, k] f32, same values the device sees
    pull_sum = (dev - _pad_correction(cents32, csum2_sk, L["npad"])) / NPIX_TOT

    push = _push_host(cents)
    loss = PUSH_W * np.mean(push) + PULL_W * pull_sum / 4.0
    return np.array(loss, dtype=F32), resA, resB


def kernel(embeddings, labels):
    loss, _, _ = run_launches(embeddings, labels, trace=False)
    return loss
